# revision 38
# baseline (speedup 1.0000x reference)
"""Trainium2 Bass kernel for nn_Attention (dense_transformer).

Reference computation (per batch n of 4):
  qkv = W_qkv @ x + b          (384, 4096)   [x flattened to (256, 64*64)]
  raw C-order reinterpret of qkv flat buffer as (4096, 384) -> q|k|v (4096,128) each
  scores = q @ k.T / 64        (4096, 4096)
  soft = softmax(scores, axis=-2)             [column softmax]
  out = soft @ v               (4096, 128)
  raw reinterpret of out as (128, 4096)
  y = W_out @ out2 + b_out     (256, 4096)

Sharding: 8 cores = 4 batches x 2 column-chunks (j-axis of the score
matrix = rows of k/v). Column-softmax stats (over i) are local to a
j-chunk; each core produces a partial y, host sums the pair.

The SPMD graph is identical on all cores; the j-half selection is encoded
host-side by rotating the qkv output channels by 192 for odd cores (which
rotates the reinterpreted sequence axis by 2048) and rotating W_out's
e-axis by 64 to compensate on the output side.

Compute layout (per core):
  stage 1: F = W_qkv @ x + b as 3 o-tiles (128, 4096) bf16 -> DRAM fbuf,
           written as half-tiles fed by ACT(lo)/DVE(hi) bias-copies.
  loads:   qT (d,i) and kT (d,j) via xbar transpose-DMA from the (4096,384)
           reinterpret view of fbuf (one batched xbar window); v (j,d) plain
           via SWDGE. All split at 512-aligned boundaries per covering
           F o-tile so phase A starts before stage 1 fully drains.
  phase A (per j-block of 128): Pt[j,i] = exp(kT_jb.T q / 64), four
           (128,1024) exps with fused column-sum accum_out; Z -> 1/Z ->
           v scaled in place. The output matmuls for hw-groups 0-3
           accumulate inline in PSUM banks 4-7 (permuted i-axis
           i' = hb*128+e via a strided rhs AP on P, software-pipelined one
           j-block behind the stats), then drain through transpose/proj2.
  phase B+C+proj2 for groups 4-7, fused per 512-wide group: 16
           accumulate-MMs -> copy -> 4 TensorE transposes -> out2g ->
           proj2 MMs + bias -> y, per-half y DMAs; banks recycle via a
           bufs=2 pool. PSUM bank g == out2 group g throughout.
"""

import numpy as np
import ml_dtypes

import concourse.bass as bass
import concourse.bacc as bacc
import concourse.mybir as mybir
from concourse.bass_utils import run_bass_kernel_spmd
from concourse.tile import TileContext, add_dep_helper
from concourse.masks import make_identity

BF16 = mybir.dt.bfloat16
F32 = mybir.dt.float32
AF = mybir.ActivationFunctionType

N, C, E, O, HW = 4, 256, 128, 384, 4096
JC = HW // 2          # j-chunk per core
NJB = JC // 128       # 16 j-blocks
SCALE = 1.0 / 64.0    # 1/sqrt(HW)

_CACHE = {}


def build_nc():
    nc = bacc.Bacc("TRN2", target_bir_lowering=False, debug=False, num_devices=8)

    x_ext = nc.dram_tensor("x", [C, HW], BF16, kind="ExternalInput").ap()
    wqkvT_ext = nc.dram_tensor("wqkvT", [C, O], BF16, kind="ExternalInput").ap()
    bqkv_ext = nc.dram_tensor("bqkv", [O, 1], F32, kind="ExternalInput").ap()
    woutT_ext = nc.dram_tensor("woutT", [E, C], BF16, kind="ExternalInput").ap()
    bout_ext = nc.dram_tensor("bout", [C, 1], F32, kind="ExternalInput").ap()
    y_ext = nc.dram_tensor("out", [C, HW], BF16, kind="ExternalOutput").ap()

    fbuf = nc.dram_tensor("fbuf", [O * HW], BF16).ap()
    fview_o = fbuf.rearrange("(o hw) -> o hw", hw=HW)   # (384, 4096) write view
    fview_i = fbuf.rearrange("(i j) -> i j", j=O)        # (4096, 384) read view

    # persistent SBUF (fixed allocations; not subject to pool slot reuse).
    # qT/kT/v are split at 512-aligned boundaries covered by successive F
    # o-tiles so phase A can start before stage 1 fully drains.
    QSPL = [0, 1024, 2560, HW]       # parts covered by F o-tiles 0/1/2
    KSPL = [0, 1024, JC]             # parts covered by F o-tiles 0/1
    VSPL = [0, 1280, JC]
    qTp = [nc.alloc_sbuf_tensor(f"qT{i}", [128, QSPL[i + 1] - QSPL[i]], BF16).ap()
           for i in range(3)]
    kTp = [nc.alloc_sbuf_tensor(f"kT{i}", [128, KSPL[i + 1] - KSPL[i]], BF16).ap()
           for i in range(2)]
    vp = [nc.alloc_sbuf_tensor(f"v{i}", [128, VSPL[i + 1] - VSPL[i]], BF16).ap()
          for i in range(2)]

    def qT_sl(i0, w=512):
        p = 0 if i0 < 1024 else (1 if i0 < 2560 else 2)
        a = i0 - QSPL[p]
        assert a + w <= QSPL[p + 1] - QSPL[p]
        return qTp[p][:, a:a + w]

    def kT_sl(jb):
        p = 0 if jb < 8 else 1
        a = jb * 128 - KSPL[p]
        return kTp[p][:, a:a + 128]

    def v_sl(jb):
        p = 0 if jb < 10 else 1
        a = jb * 128 - VSPL[p]
        return vp[p][:, a:a + 128]

    zacc = nc.alloc_sbuf_tensor("zacc", [128, 64], F32).ap()
    zsum = nc.alloc_sbuf_tensor("zsum", [128, 16], F32).ap()
    zinv = nc.alloc_sbuf_tensor("zinv", [128, 16], F32).ap()
    outT_sb = nc.alloc_sbuf_tensor("outT_sb", [128, HW], BF16).ap()
    out2g = [nc.alloc_sbuf_tensor(f"out2g{g}", [128, 512], BF16).ap()
             for g in range(8)]
    P = nc.alloc_sbuf_tensor("P", [128, NJB * HW], BF16).ap()

    with TileContext(nc) as tc:
        with tc.tile_pool(name="consts", bufs=1) as consts:
            # ---- constants (bias first: it gates the first stage-1 copies) ----
            bias = consts.tile([128, 8], F32, name="bias", tag="bias")
            bq = [bias[:, i:i + 1] for i in range(3)]
            bo = [bias[:, 3 + i:4 + i] for i in range(2)]
            for ob in range(3):
                nc.scalar.dma_start(out=bq[ob], in_=bqkv_ext[ob * 128:(ob + 1) * 128, :])
            wq_all = consts.tile([128, 2 * O], BF16, name="wq_all", tag="wq_all")
            wqT = [wq_all[:, 0:O], wq_all[:, O:2 * O]]
            for cb in range(2):
                nc.scalar.dma_start(out=wqT[cb], in_=wqkvT_ext[cb * 128:(cb + 1) * 128, :])
            for cb in range(2):
                nc.scalar.dma_start(out=bo[cb], in_=bout_ext[cb * 128:(cb + 1) * 128, :])
            misc = consts.tile([128, C + 128], BF16, name="misc", tag="misc")
            woutT = misc[:, 0:C]
            ident = misc[:, C:C + 128]
            nc.scalar.dma_start(out=woutT, in_=woutT_ext[:])
            make_identity(nc, ident)
            scratch = consts.tile([128, 1], F32, name="scratch", tag="scratch")
            nc.vector.memset(scratch[:], 0.0)
            nc.scalar.activation(scratch[:], scratch[:], AF.Exp)

            # ---- PE warmup: dummy matmuls so HAM is at full clock before
            #      stage 1 (identity data; output never read) ----
            wsrc = consts.tile([128, 128], BF16, name="wsrc", tag="wsrc")
            nc.vector.memset(wsrc[:], 1.0)
            with tc.tile_pool(name="psW", bufs=1, space="PSUM") as psW:
                wtile = psW.tile([128, 128], F32, tag="warm")
                for _ in range(16):
                    nc.tensor.matmul(wtile[:], wsrc[:], wsrc[:], start=True, stop=True)

            # ---- x loads (2 x 1MB, sync ring) ----
            early = tc.alloc_tile_pool(name="early", bufs=1)
            # x split (cb, half) so the first matmuls start after 2 chunks
            xsb = [[early.tile([128, HW // 2], BF16, name=f"x{cb}{h}", tag=f"x{cb}{h}")
                    for h in range(2)] for cb in range(2)]
            Fsb = [[early.tile([128, HW // 2], BF16, name=f"F{i}{hh}", tag=f"F{i}{hh}")
                    for hh in range(2)] for i in range(3)]
            for h in range(2):
                for cb in range(2):
                    nc.sync.dma_start(
                        out=xsb[cb][h][:],
                        in_=x_ext[cb * 128:(cb + 1) * 128,
                                  h * (HW // 2):(h + 1) * (HW // 2)])

            # ---- stage 1: qkv projection -> Fsb o-tiles -> fbuf,
            #      with q/k/v part-loads woven in right after each F write ----
            with tc.tile_pool(name="psF", bufs=4, space="PSUM") as psF:
                f_writes = []
                vlds = []
                for ob in range(3):
                    for nch in range(8):
                        pf = psF.tile([128, 512], F32, tag="pf")
                        h, o512 = nch // 4, (nch % 4) * 512
                        sl = slice(nch * 512, (nch + 1) * 512)
                        nc.tensor.matmul(
                            pf[:], wqT[0][:, ob * 128:(ob + 1) * 128],
                            xsb[0][h][:, o512:o512 + 512],
                            start=True, stop=False,
                        )
                        nc.tensor.matmul(
                            pf[:], wqT[1][:, ob * 128:(ob + 1) * 128],
                            xsb[1][h][:, o512:o512 + 512],
                            start=False, stop=True,
                        )
                        fb = Fsb[ob][nch // 4]
                        fsl = slice((nch % 4) * 512, (nch % 4 + 1) * 512)
                        if nch < 4:
                            nc.vector.tensor_scalar_add(fb[:, fsl], pf[:], bq[ob])
                        else:
                            nc.scalar.activation(fb[:, fsl], pf[:], AF.Identity,
                                                 bias=bq[ob])
                    ws = []
                    for hh in range(2):
                        eng = nc.sync if hh == 0 else nc.scalar
                        ws.append(eng.dma_start(
                            out=fview_o[ob * 128:(ob + 1) * 128,
                                        hh * (HW // 2):(hh + 1) * (HW // 2)],
                            in_=Fsb[ob][hh][:],
                        ))
                    f_writes.append(ws)
                    # v loads (plain copies, scalar ring) right after their F write
                    if ob == 0:
                        r = nc.gpsimd.dma_start(
                            out=vp[0].rearrange("p (t d) -> p t d", d=128),
                            in_=fview_i[0:1280, 2 * E:3 * E].rearrange(
                                "(t p) d -> p t d", p=128))
                        vlds.append(r)
                        for w2 in ws:
                            add_dep_helper(r.ins, w2.ins, reason="fbuf RAW")
                    elif ob == 1:
                        r = nc.gpsimd.dma_start(
                            out=vp[1].rearrange("p (t d) -> p t d", d=128),
                            in_=fview_i[1280:2048, 2 * E:3 * E].rearrange(
                                "(t p) d -> p t d", p=128))
                        vlds.append(r)
                        for w2 in ws:
                            add_dep_helper(r.ins, w2.ins, reason="fbuf RAW")
                # all xbar transposes batched in one mode-window (sync ring).
                # The first three only order behind F0/F1 so they run before
                # F2's writes; v loads are pushed behind the transposes in the
                # bandwidth queue (v is not needed until the first stats).
                tr_specs = [
                    (qTp[0], fview_i[0:1024, 0:E], 0, 2),
                    (kTp[0], fview_i[0:1024, E:2 * E], 0, 2),
                    (qTp[1], fview_i[1024:2560, 0:E], 1, 2),
                    (qTp[2], fview_i[2560:HW, 0:E], 2, 3),
                    (kTp[1], fview_i[1024:2048, E:2 * E], 1, 3),
                ]
                trs = []
                for dst, srcap, dep, nhint in tr_specs:
                    rt = nc.sync.dma_start_transpose(out=dst[:], in_=srcap)
                    trs.append(rt)
                    for w2 in f_writes[dep]:
                        add_dep_helper(rt.ins, w2.ins, reason="fbuf RAW")
                    for ws2 in f_writes[:nhint]:
                        for w2 in ws2:
                            add_dep_helper(rt.ins, w2.ins, sync=False,
                                           reason="xbar window after copies")
            early.release()

            # ---- keep PE warm across the load window: dummy-MM waves
            #      keyed to F-write completions so HAM never re-throttles ----
            with tc.tile_pool(name="psW2", bufs=1, space="PSUM") as psW2:
                w2t = psW2.tile([128, 128], F32, tag="warm2")
                for wv, ws2 in enumerate(f_writes):
                    mm0 = nc.tensor.matmul(w2t[:], wsrc[:], wsrc[:],
                                           start=True, stop=True)
                    add_dep_helper(mm0.ins, ws2[0].ins, sync=True,
                                   reason="warm wave pacing")
                    for _ in range(5):
                        nc.tensor.matmul(w2t[:], wsrc[:], wsrc[:],
                                         start=True, stop=True)

            # ---- phase A: scores + exp(1024-wide, fused column sums),
            #      with groups 0-3 of the output matmul inlined (banks 4-7),
            #      software-pipelined one j-block behind the stats ----
            P3 = P.rearrange("p (jb e hb) -> p jb hb e", jb=NJB, hb=32)
            with tc.tile_pool(name="psBi", bufs=1, space="PSUM") as psBi:
                obi = [psBi.tile([128, 512], F32, name=f"obi{g}", tag=f"obi{g}")
                       for g in range(4)]

                def inline_mms(jb):
                    for g in range(4):
                        nc.tensor.matmul(
                            obi[g][:], v_sl(jb), P3[:, jb, 4 * g:4 * g + 4, :],
                            start=(jb == 0), stop=(jb == NJB - 1),
                        )

                with tc.tile_pool(name="psA", bufs=2, space="PSUM") as psA:
                    def score_exp(jb, h):
                        pa = psA.tile([128, 1024], F32, tag="pa")
                        for n2 in range(2):
                            i0 = h * 1024 + n2 * 512
                            nc.tensor.matmul(
                                pa[:, n2 * 512:(n2 + 1) * 512],
                                kT_sl(jb), qT_sl(i0),
                                start=True, stop=True,
                            )
                        nc.scalar.activation(
                            out=P[:, jb * HW + h * 1024: jb * HW + (h + 1) * 1024],
                            in_=pa[:],
                            func=AF.Exp,
                            scale=SCALE,
                            accum_out=zacc[:, jb * 4 + h: jb * 4 + h + 1],
                        )

                    # h0/h1 only touch qT parts 0-1; h2/h3 need part 2 which
                    # lands last. Lead with h0/h1 of the first three j-blocks
                    # so the exp chain stays dense while qT2 is in flight.
                    LEAD = 3
                    for jb in range(LEAD):
                        score_exp(jb, 0)
                    for jb in range(LEAD):
                        score_exp(jb, 1)
                    for jb in range(NJB):
                        score_exp(jb, 2)
                        score_exp(jb, 3)
                        nc.vector.reduce_sum(
                            out=zsum[:, jb:jb + 1],
                            in_=zacc[:, jb * 4:(jb + 1) * 4],
                            axis=mybir.AxisListType.X,
                        )
                        nc.vector.reciprocal(zinv[:, jb:jb + 1], zsum[:, jb:jb + 1])
                        nc.vector.tensor_scalar_mul(
                            v_sl(jb), v_sl(jb), zinv[:, jb:jb + 1],
                        )
                        if jb + LEAD < NJB:
                            score_exp(jb + LEAD, 0)
                            score_exp(jb + LEAD, 1)
                        if jb > 1:
                            inline_mms(jb - 2)
                    inline_mms(NJB - 2)
                    inline_mms(NJB - 1)

                # drain inline groups 0-3 through transpose/proj2 (C-part only)
                with tc.tile_pool(name="psC0", bufs=2, space="PSUM") as psC0, \
                     tc.tile_pool(name="psY0", bufs=2, space="PSUM") as psY0, \
                     tc.tile_pool(name="late0", bufs=1) as late0:
                    ysb0h = [late0.tile([128, HW // 2], BF16, name=f"yh{cb}",
                                        tag=f"yh{cb}") for cb in range(2)]
                    for g in range(4):
                        gsl = slice(g * 512, (g + 1) * 512)
                        if g % 2 == 0:
                            nc.scalar.copy(outT_sb[:, gsl], obi[g][:])
                        else:
                            nc.vector.tensor_copy(outT_sb[:, gsl], obi[g][:])
                        tp = psC0.tile([128, 512], BF16, tag="tp0")
                        for s in range(4):
                            nc.tensor.transpose(
                                tp[:, s * 128:(s + 1) * 128],
                                outT_sb[:, g * 512 + s * 128: g * 512 + (s + 1) * 128],
                                ident,
                            )
                        if g % 2 == 0:
                            nc.vector.tensor_copy(out2g[g][:], tp[:])
                        else:
                            nc.scalar.copy(out2g[g][:], tp[:])
                        gof = (g % 4) * 512
                        for cb in range(2):
                            py = psY0.tile([128, 512], F32, tag="py0")
                            nc.tensor.matmul(
                                py[:], woutT[:, cb * 128:(cb + 1) * 128], out2g[g][:],
                                start=True, stop=True,
                            )
                            dst = ysb0h[cb][:, gof:gof + 512]
                            if cb == 0:
                                nc.scalar.activation(dst, py[:], AF.Identity,
                                                     bias=bo[cb])
                            else:
                                nc.vector.tensor_scalar_add(dst, py[:], bo[cb])
                        if g == 3:
                            for cb in range(2):
                                [nc.sync, nc.scalar][cb].dma_start(
                                    out=y_ext[cb * 128:(cb + 1) * 128, 0:2048],
                                    in_=ysb0h[cb][:])

            # ---- phase B + C + proj2, fused per 512-wide group ----
            # outT is produced with permuted i-axis: i' = hb*128 + e (hb = hw
            # block, e = embed row), so PSUM bank g holds exactly the data for
            # out2 group g: transpose outT'[:, hb*128:+128].T = out2[:, hb*128:+128].
            # The permutation comes free via a strided rhs AP on P.
            with tc.tile_pool(name="psB", bufs=2, space="PSUM") as psB, \
                 tc.tile_pool(name="psC", bufs=2, space="PSUM") as psC, \
                 tc.tile_pool(name="psY", bufs=3, space="PSUM") as psY, \
                 tc.tile_pool(name="late", bufs=1) as late:
                ysb = [[late.tile([128, HW // 2], BF16, name=f"y{cb}{hh}",
                                  tag=f"y{cb}{hh}") for hh in range(2)]
                       for cb in range(2)]
                for g in range(4, 8):
                    ob_ps = psB.tile([128, 512], F32, tag="ob_ps")
                    for jb in range(NJB):
                        nc.tensor.matmul(
                            ob_ps[:],
                            v_sl(jb),
                            P3[:, jb, 4 * g:4 * g + 4, :],
                            start=(jb == 0), stop=(jb == NJB - 1),
                        )
                    gsl = slice(g * 512, (g + 1) * 512)
                    if g % 2 == 0:
                        nc.scalar.copy(outT_sb[:, gsl], ob_ps[:])
                    else:
                        nc.vector.tensor_copy(outT_sb[:, gsl], ob_ps[:])
                    tp = psC.tile([128, 512], BF16, tag="tp")
                    for s in range(4):
                        nc.tensor.transpose(
                            tp[:, s * 128:(s + 1) * 128],
                            outT_sb[:, g * 512 + s * 128: g * 512 + (s + 1) * 128],
                            ident,
                        )
                    if g % 2 == 0:
                        nc.vector.tensor_copy(out2g[g][:], tp[:])
                    else:
                        nc.scalar.copy(out2g[g][:], tp[:])
                    hh, gof = g // 4, (g % 4) * 512
                    for cb in range(2):
                        py = psY.tile([128, 512], F32, tag="py")
                        nc.tensor.matmul(
                            py[:], woutT[:, cb * 128:(cb + 1) * 128], out2g[g][:],
                            start=True, stop=True,
                        )
                        dst = ysb[cb][hh][:, gof:gof + 512]
                        if cb == 0:
                            nc.scalar.activation(dst, py[:], AF.Identity, bias=bo[cb])
                        else:
                            nc.vector.tensor_scalar_add(dst, py[:], bo[cb])
                    if g == 7:
                        for cb in range(2):
                            [nc.sync, nc.scalar][cb].dma_start(
                                out=y_ext[cb * 128:(cb + 1) * 128, 2048:4096],
                                in_=ysb[cb][1][:])

    nc.compile()
    return nc


def get_nc():
    if "nc" not in _CACHE:
        _CACHE["nc"] = build_nc()
    return _CACHE["nc"]


def make_in_maps(x, W_qkv, b_qkv, W_out, b_out):
    x = np.asarray(x, dtype=np.float32)
    W_qkv = np.asarray(W_qkv, dtype=np.float32)
    b_qkv = np.asarray(b_qkv, dtype=np.float32)
    W_out = np.asarray(W_out, dtype=np.float32)
    b_out = np.asarray(b_out, dtype=np.float32)

    operm = (np.arange(O) + O // 2) % O      # rotate qkv channels by 192
    eperm = (np.arange(E) + E // 2) % E      # rotate e-axis by 64

    halves = []
    for h in range(2):
        if h == 0:
            wq, bqv, wo, bov = W_qkv, b_qkv, W_out, b_out
        else:
            wq = W_qkv[operm]
            bqv = b_qkv[operm]
            wo = W_out[:, eperm]
            bov = np.zeros_like(b_out)
        halves.append({
            "wqkvT": np.ascontiguousarray(wq.T).astype(ml_dtypes.bfloat16),
            "bqkv": np.ascontiguousarray(bqv.reshape(O, 1)),
            "woutT": np.ascontiguousarray(wo.T).astype(ml_dtypes.bfloat16),
            "bout": np.ascontiguousarray(bov.reshape(C, 1)),
        })

    xb = [np.ascontiguousarray(x[n].reshape(C, HW)).astype(ml_dtypes.bfloat16)
          for n in range(N)]
    in_maps = []
    for core in range(8):
        n, h = core // 2, core % 2
        m = {"x": xb[n]}
        m.update(halves[h])
        in_maps.append(m)
    return in_maps


def run(inputs, trace=False, **kw):
    nc = get_nc()
    in_maps = make_in_maps(**inputs)
    res = run_bass_kernel_spmd(nc, in_maps, core_ids=list(range(8)), trace=trace, **kw)
    ys = [np.asarray(res.results[i]["out"], dtype=np.float32) for i in range(8)]
    y = np.stack([ys[2 * n] + ys[2 * n + 1] for n in range(N)])
    return y.reshape(N, C, 64, 64), res


def kernel(**inputs):
    y, _ = run(inputs, trace=False)
    return y


# revision 42
# speedup vs baseline: 1.0101x; 1.0101x over previous
"""Trainium2 Bass kernel for nn_Attention (dense_transformer).

Reference computation (per batch n of 4):
  qkv = W_qkv @ x + b          (384, 4096)   [x flattened to (256, 64*64)]
  raw C-order reinterpret of qkv flat buffer as (4096, 384) -> q|k|v (4096,128) each
  scores = q @ k.T / 64        (4096, 4096)
  soft = softmax(scores, axis=-2)             [column softmax]
  out = soft @ v               (4096, 128)
  raw reinterpret of out as (128, 4096)
  y = W_out @ out2 + b_out     (256, 4096)

Sharding: 8 cores = 4 batches x 2 column-chunks (j-axis of the score
matrix = rows of k/v). Column-softmax stats (over i) are local to a
j-chunk; each core produces a partial y, host sums the pair.

The SPMD graph is identical on all cores; the j-half selection is encoded
host-side by rotating the qkv output channels by 192 for odd cores (which
rotates the reinterpreted sequence axis by 2048) and rotating W_out's
e-axis by 64 to compensate on the output side.

Compute layout (per core):
  stage 1: F = W_qkv @ x + b as 3 o-tiles (128, 4096) bf16 -> DRAM fbuf,
           written as half-tiles fed by ACT(lo)/DVE(hi) bias-copies.
  loads:   qT (d,i) and kT (d,j) via xbar transpose-DMA from the (4096,384)
           reinterpret view of fbuf (one batched xbar window); v (j,d) plain
           via SWDGE. All split at 512-aligned boundaries per covering
           F o-tile so phase A starts before stage 1 fully drains.
  phase A (per j-block of 128): Pt[j,i] = exp(kT_jb.T q / 64), four
           (128,1024) exps with fused column-sum accum_out; Z -> 1/Z ->
           v scaled in place. The output matmuls for hw-groups 0-3
           accumulate inline in PSUM banks 4-7 (permuted i-axis
           i' = hb*128+e via a strided rhs AP on P, software-pipelined one
           j-block behind the stats), then drain through transpose/proj2.
  phase B+C+proj2 for groups 4-7, fused per 512-wide group: 16
           accumulate-MMs -> copy -> 4 TensorE transposes -> out2g ->
           proj2 MMs + bias -> y, per-half y DMAs; banks recycle via a
           bufs=2 pool. PSUM bank g == out2 group g throughout.
"""

import numpy as np
import ml_dtypes

import concourse.bass as bass
import concourse.bacc as bacc
import concourse.mybir as mybir
from concourse.bass_utils import run_bass_kernel_spmd
from concourse.tile import TileContext, add_dep_helper
from concourse.masks import make_identity

BF16 = mybir.dt.bfloat16
F32 = mybir.dt.float32
AF = mybir.ActivationFunctionType

N, C, E, O, HW = 4, 256, 128, 384, 4096
JC = HW // 2          # j-chunk per core
NJB = JC // 128       # 16 j-blocks
SCALE = 1.0 / 64.0    # 1/sqrt(HW)

_CACHE = {}


def build_nc():
    nc = bacc.Bacc("TRN2", target_bir_lowering=False, debug=False, num_devices=8)

    x_ext = nc.dram_tensor("x", [C, HW], BF16, kind="ExternalInput").ap()
    wqkvT_ext = nc.dram_tensor("wqkvT", [C, O], BF16, kind="ExternalInput").ap()
    bqkv_ext = nc.dram_tensor("bqkv", [O, 1], F32, kind="ExternalInput").ap()
    woutT_ext = nc.dram_tensor("woutT", [E, C], BF16, kind="ExternalInput").ap()
    bout_ext = nc.dram_tensor("bout", [C, 1], F32, kind="ExternalInput").ap()
    y_ext = nc.dram_tensor("out", [C, HW], BF16, kind="ExternalOutput").ap()

    fbuf = nc.dram_tensor("fbuf", [O * HW], BF16).ap()
    fview_o = fbuf.rearrange("(o hw) -> o hw", hw=HW)   # (384, 4096) write view
    fview_i = fbuf.rearrange("(i j) -> i j", j=O)        # (4096, 384) read view

    # persistent SBUF (fixed allocations; not subject to pool slot reuse).
    # qT/kT/v are split at 512-aligned boundaries covered by successive F
    # o-tiles so phase A can start before stage 1 fully drains.
    QSPL = [0, 1024, 2560, HW]       # parts covered by F o-tiles 0/1/2
    KSPL = [0, 1024, JC]             # parts covered by F o-tiles 0/1
    VSPL = [0, 1280, JC]
    qTp = [nc.alloc_sbuf_tensor(f"qT{i}", [128, QSPL[i + 1] - QSPL[i]], BF16).ap()
           for i in range(3)]
    kTp = [nc.alloc_sbuf_tensor(f"kT{i}", [128, KSPL[i + 1] - KSPL[i]], BF16).ap()
           for i in range(2)]
    vp = [nc.alloc_sbuf_tensor(f"v{i}", [128, VSPL[i + 1] - VSPL[i]], BF16).ap()
          for i in range(2)]

    def qT_sl(i0, w=512):
        p = 0 if i0 < 1024 else (1 if i0 < 2560 else 2)
        a = i0 - QSPL[p]
        assert a + w <= QSPL[p + 1] - QSPL[p]
        return qTp[p][:, a:a + w]

    def kT_sl(jb):
        p = 0 if jb < 8 else 1
        a = jb * 128 - KSPL[p]
        return kTp[p][:, a:a + 128]

    def v_sl(jb):
        p = 0 if jb < 10 else 1
        a = jb * 128 - VSPL[p]
        return vp[p][:, a:a + 128]

    zacc = nc.alloc_sbuf_tensor("zacc", [128, 64], F32).ap()
    zsum = nc.alloc_sbuf_tensor("zsum", [128, 16], F32).ap()
    zinv = nc.alloc_sbuf_tensor("zinv", [128, 16], F32).ap()
    outT_sb = nc.alloc_sbuf_tensor("outT_sb", [128, HW], BF16).ap()
    out2g = [nc.alloc_sbuf_tensor(f"out2g{g}", [128, 512], BF16).ap()
             for g in range(8)]
    P = nc.alloc_sbuf_tensor("P", [128, NJB * HW], BF16).ap()

    with TileContext(nc) as tc:
        with tc.tile_pool(name="consts", bufs=1) as consts:
            # ---- constants (bias first: it gates the first stage-1 copies) ----
            bias = consts.tile([128, 8], F32, name="bias", tag="bias")
            bq = [bias[:, i:i + 1] for i in range(3)]
            bo = [bias[:, 3 + i:4 + i] for i in range(2)]
            for ob in range(3):
                nc.scalar.dma_start(out=bq[ob], in_=bqkv_ext[ob * 128:(ob + 1) * 128, :])
            wq_all = consts.tile([128, 2 * O], BF16, name="wq_all", tag="wq_all")
            wqT = [wq_all[:, 0:O], wq_all[:, O:2 * O]]
            for cb in range(2):
                nc.scalar.dma_start(out=wqT[cb], in_=wqkvT_ext[cb * 128:(cb + 1) * 128, :])
            for cb in range(2):
                nc.scalar.dma_start(out=bo[cb], in_=bout_ext[cb * 128:(cb + 1) * 128, :])
            misc = consts.tile([128, C + 128], BF16, name="misc", tag="misc")
            woutT = misc[:, 0:C]
            ident = misc[:, C:C + 128]
            nc.scalar.dma_start(out=woutT, in_=woutT_ext[:])
            make_identity(nc, ident)
            scratch = consts.tile([128, 1], F32, name="scratch", tag="scratch")
            nc.vector.memset(scratch[:], 0.0)
            nc.scalar.activation(scratch[:], scratch[:], AF.Exp)

            # ---- PE warmup: dummy matmuls so HAM is at full clock before
            #      stage 1 (identity data; output never read) ----
            wsrc = consts.tile([128, 128], BF16, name="wsrc", tag="wsrc")
            nc.vector.memset(wsrc[:], 1.0)
            with tc.tile_pool(name="psW", bufs=1, space="PSUM") as psW:
                wtile = psW.tile([128, 128], F32, tag="warm")
                for _ in range(16):
                    nc.tensor.matmul(wtile[:], wsrc[:], wsrc[:], start=True, stop=True)

            # ---- x loads (2 x 1MB, sync ring) ----
            early = tc.alloc_tile_pool(name="early", bufs=1)
            # x split (cb, half) so the first matmuls start after 2 chunks
            xsb = [[early.tile([128, HW // 2], BF16, name=f"x{cb}{h}", tag=f"x{cb}{h}")
                    for h in range(2)] for cb in range(2)]
            Fsb = [[early.tile([128, HW // 2], BF16, name=f"F{i}{hh}", tag=f"F{i}{hh}")
                    for hh in range(2)] for i in range(3)]
            for h in range(2):
                for cb in range(2):
                    nc.sync.dma_start(
                        out=xsb[cb][h][:],
                        in_=x_ext[cb * 128:(cb + 1) * 128,
                                  h * (HW // 2):(h + 1) * (HW // 2)])

            # ---- stage 1: qkv projection -> Fsb o-tiles -> fbuf,
            #      with q/k/v part-loads woven in right after each F write ----
            with tc.tile_pool(name="psF", bufs=4, space="PSUM") as psF:
                f_writes = []
                vlds = []
                for ob in range(3):
                    for nch in range(8):
                        pf = psF.tile([128, 512], F32, tag="pf")
                        h, o512 = nch // 4, (nch % 4) * 512
                        sl = slice(nch * 512, (nch + 1) * 512)
                        nc.tensor.matmul(
                            pf[:], wqT[0][:, ob * 128:(ob + 1) * 128],
                            xsb[0][h][:, o512:o512 + 512],
                            start=True, stop=False,
                        )
                        nc.tensor.matmul(
                            pf[:], wqT[1][:, ob * 128:(ob + 1) * 128],
                            xsb[1][h][:, o512:o512 + 512],
                            start=False, stop=True,
                        )
                        fb = Fsb[ob][nch // 4]
                        fsl = slice((nch % 4) * 512, (nch % 4 + 1) * 512)
                        if nch < 4:
                            nc.vector.tensor_scalar_add(fb[:, fsl], pf[:], bq[ob])
                        else:
                            nc.scalar.activation(fb[:, fsl], pf[:], AF.Identity,
                                                 bias=bq[ob])
                    ws = []
                    for hh in range(2):
                        eng = nc.sync if hh == 0 else nc.scalar
                        ws.append(eng.dma_start(
                            out=fview_o[ob * 128:(ob + 1) * 128,
                                        hh * (HW // 2):(hh + 1) * (HW // 2)],
                            in_=Fsb[ob][hh][:],
                        ))
                    f_writes.append(ws)
                    # v loads (plain copies, scalar ring) right after their F write
                    if ob == 0:
                        r = nc.gpsimd.dma_start(
                            out=vp[0].rearrange("p (t d) -> p t d", d=128),
                            in_=fview_i[0:1280, 2 * E:3 * E].rearrange(
                                "(t p) d -> p t d", p=128))
                        vlds.append(r)
                        for w2 in ws:
                            add_dep_helper(r.ins, w2.ins, reason="fbuf RAW")
                    elif ob == 1:
                        r = nc.gpsimd.dma_start(
                            out=vp[1].rearrange("p (t d) -> p t d", d=128),
                            in_=fview_i[1280:2048, 2 * E:3 * E].rearrange(
                                "(t p) d -> p t d", p=128))
                        vlds.append(r)
                        for w2 in ws:
                            add_dep_helper(r.ins, w2.ins, reason="fbuf RAW")
                # all xbar transposes batched in one mode-window (sync ring).
                # The first three only order behind F0/F1 so they run before
                # F2's writes; v loads are pushed behind the transposes in the
                # bandwidth queue (v is not needed until the first stats).
                tr_specs = [
                    (qTp[0], fview_i[0:1024, 0:E], 0, 2),
                    (kTp[0], fview_i[0:1024, E:2 * E], 0, 2),
                    (qTp[1], fview_i[1024:2560, 0:E], 1, 2),
                    (qTp[2], fview_i[2560:HW, 0:E], 2, 3),
                    (kTp[1], fview_i[1024:2048, E:2 * E], 1, 3),
                ]
                trs = []
                for dst, srcap, dep, nhint in tr_specs:
                    rt = nc.sync.dma_start_transpose(out=dst[:], in_=srcap)
                    trs.append(rt)
                    for w2 in f_writes[dep]:
                        add_dep_helper(rt.ins, w2.ins, reason="fbuf RAW")
                    for ws2 in f_writes[:nhint]:
                        for w2 in ws2:
                            add_dep_helper(rt.ins, w2.ins, sync=False,
                                           reason="xbar window after copies")
            early.release()

            # ---- keep PE warm across the load window: dummy-MM waves
            #      keyed to F-write completions so HAM never re-throttles ----
            with tc.tile_pool(name="psW2", bufs=1, space="PSUM") as psW2:
                w2t = psW2.tile([128, 128], F32, tag="warm2")
                for wv, ws2 in enumerate(f_writes):
                    mm0 = nc.tensor.matmul(w2t[:], wsrc[:], wsrc[:],
                                           start=True, stop=True)
                    add_dep_helper(mm0.ins, ws2[0].ins, sync=True,
                                   reason="warm wave pacing")
                    for _ in range(5):
                        nc.tensor.matmul(w2t[:], wsrc[:], wsrc[:],
                                         start=True, stop=True)

            # ---- phase A: scores + exp(1024-wide, fused column sums),
            #      with groups 0-3 of the output matmul inlined (banks 4-7),
            #      software-pipelined one j-block behind the stats ----
            P3 = P.rearrange("p (jb e hb) -> p jb hb e", jb=NJB, hb=32)
            with tc.tile_pool(name="psBi", bufs=1, space="PSUM") as psBi:
                obi = [psBi.tile([128, 512], F32, name=f"obi{g}", tag=f"obi{g}")
                       for g in range(4)]

                def inline_mms(jb):
                    for g in range(4):
                        nc.tensor.matmul(
                            obi[g][:], v_sl(jb), P3[:, jb, 4 * g:4 * g + 4, :],
                            start=(jb == 0), stop=(jb == NJB - 1),
                        )

                with tc.tile_pool(name="psA", bufs=2, space="PSUM") as psA:
                    def score_exp(jb, h):
                        pa = psA.tile([128, 1024], F32, tag="pa")
                        for n2 in range(2):
                            i0 = h * 1024 + n2 * 512
                            nc.tensor.matmul(
                                pa[:, n2 * 512:(n2 + 1) * 512],
                                kT_sl(jb), qT_sl(i0),
                                start=True, stop=True,
                            )
                        nc.scalar.activation(
                            out=P[:, jb * HW + h * 1024: jb * HW + (h + 1) * 1024],
                            in_=pa[:],
                            func=AF.Exp,
                            scale=SCALE,
                            accum_out=zacc[:, jb * 4 + h: jb * 4 + h + 1],
                        )

                    # h0/h1 only touch qT parts 0-1; h2/h3 need part 2 which
                    # lands last. Lead with h0/h1 of the first three j-blocks
                    # so the exp chain stays dense while qT2 is in flight.
                    LEAD = 3
                    for jb in range(LEAD):
                        score_exp(jb, 0)
                    for jb in range(LEAD):
                        score_exp(jb, 1)
                    for jb in range(NJB):
                        score_exp(jb, 2)
                        score_exp(jb, 3)
                        nc.vector.reduce_sum(
                            out=zsum[:, jb:jb + 1],
                            in_=zacc[:, jb * 4:(jb + 1) * 4],
                            axis=mybir.AxisListType.X,
                        )
                        nc.vector.reciprocal(zinv[:, jb:jb + 1], zsum[:, jb:jb + 1])
                        nc.vector.tensor_scalar_mul(
                            v_sl(jb), v_sl(jb), zinv[:, jb:jb + 1],
                        )
                        if jb + LEAD < NJB:
                            score_exp(jb + LEAD, 0)
                            score_exp(jb + LEAD, 1)
                        if jb > 1:
                            inline_mms(jb - 2)
                    inline_mms(NJB - 2)
                    inline_mms(NJB - 1)

                # drain inline groups 0-3 through transpose/proj2 (C-part only)
                with tc.tile_pool(name="psC0", bufs=2, space="PSUM") as psC0, \
                     tc.tile_pool(name="psY0", bufs=2, space="PSUM") as psY0, \
                     tc.tile_pool(name="late0", bufs=1) as late0:
                    yg0 = [[late0.tile([128, 512], BF16, name=f"yg{cb}_{g}",
                                       tag=f"yg{cb}_{g}") for g in range(4)]
                           for cb in range(2)]
                    for g in range(4):
                        gsl = slice(g * 512, (g + 1) * 512)
                        if g % 2 == 0:
                            nc.scalar.copy(outT_sb[:, gsl], obi[g][:])
                        else:
                            nc.vector.tensor_copy(outT_sb[:, gsl], obi[g][:])
                        tp = psC0.tile([128, 512], BF16, tag="tp0")
                        for s in range(4):
                            nc.tensor.transpose(
                                tp[:, s * 128:(s + 1) * 128],
                                outT_sb[:, g * 512 + s * 128: g * 512 + (s + 1) * 128],
                                ident,
                            )
                        if g % 2 == 0:
                            nc.vector.tensor_copy(out2g[g][:], tp[:])
                        else:
                            nc.scalar.copy(out2g[g][:], tp[:])
                        for cb in range(2):
                            py = psY0.tile([128, 512], F32, tag="py0")
                            nc.tensor.matmul(
                                py[:], woutT[:, cb * 128:(cb + 1) * 128], out2g[g][:],
                                start=True, stop=True,
                            )
                            dst = yg0[cb][g][:]
                            if cb == 0:
                                nc.scalar.activation(dst, py[:], AF.Identity,
                                                     bias=bo[cb])
                            else:
                                nc.vector.tensor_scalar_add(dst, py[:], bo[cb])
                            [nc.sync, nc.scalar][cb].dma_start(
                                out=y_ext[cb * 128:(cb + 1) * 128,
                                          g * 512:(g + 1) * 512],
                                in_=dst)

            # ---- phase B + C + proj2, fused per 512-wide group ----
            # outT is produced with permuted i-axis: i' = hb*128 + e (hb = hw
            # block, e = embed row), so PSUM bank g holds exactly the data for
            # out2 group g: transpose outT'[:, hb*128:+128].T = out2[:, hb*128:+128].
            # The permutation comes free via a strided rhs AP on P.
            with tc.tile_pool(name="psB", bufs=2, space="PSUM") as psB, \
                 tc.tile_pool(name="psC", bufs=2, space="PSUM") as psC, \
                 tc.tile_pool(name="psY", bufs=3, space="PSUM") as psY, \
                 tc.tile_pool(name="late", bufs=1) as late:
                yg1 = [[late.tile([128, 512], BF16, name=f"yb{cb}_{g}",
                                  tag=f"yb{cb}_{g}") for g in range(4)]
                       for cb in range(2)]
                for g in range(4, 8):
                    ob_ps = psB.tile([128, 512], F32, tag="ob_ps")
                    for jb in range(NJB):
                        nc.tensor.matmul(
                            ob_ps[:],
                            v_sl(jb),
                            P3[:, jb, 4 * g:4 * g + 4, :],
                            start=(jb == 0), stop=(jb == NJB - 1),
                        )
                    gsl = slice(g * 512, (g + 1) * 512)
                    if g % 2 == 0:
                        nc.scalar.copy(outT_sb[:, gsl], ob_ps[:])
                    else:
                        nc.vector.tensor_copy(outT_sb[:, gsl], ob_ps[:])
                    tp = psC.tile([128, 512], BF16, tag="tp")
                    for s in range(4):
                        nc.tensor.transpose(
                            tp[:, s * 128:(s + 1) * 128],
                            outT_sb[:, g * 512 + s * 128: g * 512 + (s + 1) * 128],
                            ident,
                        )
                    if g % 2 == 0:
                        nc.vector.tensor_copy(out2g[g][:], tp[:])
                    else:
                        nc.scalar.copy(out2g[g][:], tp[:])
                    for cb in range(2):
                        py = psY.tile([128, 512], F32, tag="py")
                        nc.tensor.matmul(
                            py[:], woutT[:, cb * 128:(cb + 1) * 128], out2g[g][:],
                            start=True, stop=True,
                        )
                        dst = yg1[cb][g - 4][:]
                        if cb == 0:
                            nc.scalar.activation(dst, py[:], AF.Identity, bias=bo[cb])
                        else:
                            nc.vector.tensor_scalar_add(dst, py[:], bo[cb])
                        [nc.sync, nc.scalar][cb].dma_start(
                            out=y_ext[cb * 128:(cb + 1) * 128,
                                      g * 512:(g + 1) * 512],
                            in_=dst)

    nc.compile()
    return nc


def get_nc():
    if "nc" not in _CACHE:
        _CACHE["nc"] = build_nc()
    return _CACHE["nc"]


def make_in_maps(x, W_qkv, b_qkv, W_out, b_out):
    x = np.asarray(x, dtype=np.float32)
    W_qkv = np.asarray(W_qkv, dtype=np.float32)
    b_qkv = np.asarray(b_qkv, dtype=np.float32)
    W_out = np.asarray(W_out, dtype=np.float32)
    b_out = np.asarray(b_out, dtype=np.float32)

    operm = (np.arange(O) + O // 2) % O      # rotate qkv channels by 192
    eperm = (np.arange(E) + E // 2) % E      # rotate e-axis by 64

    halves = []
    for h in range(2):
        if h == 0:
            wq, bqv, wo, bov = W_qkv, b_qkv, W_out, b_out
        else:
            wq = W_qkv[operm]
            bqv = b_qkv[operm]
            wo = W_out[:, eperm]
            bov = np.zeros_like(b_out)
        halves.append({
            "wqkvT": np.ascontiguousarray(wq.T).astype(ml_dtypes.bfloat16),
            "bqkv": np.ascontiguousarray(bqv.reshape(O, 1)),
            "woutT": np.ascontiguousarray(wo.T).astype(ml_dtypes.bfloat16),
            "bout": np.ascontiguousarray(bov.reshape(C, 1)),
        })

    xb = [np.ascontiguousarray(x[n].reshape(C, HW)).astype(ml_dtypes.bfloat16)
          for n in range(N)]
    in_maps = []
    for core in range(8):
        n, h = core // 2, core % 2
        m = {"x": xb[n]}
        m.update(halves[h])
        in_maps.append(m)
    return in_maps


def run(inputs, trace=False, **kw):
    nc = get_nc()
    in_maps = make_in_maps(**inputs)
    res = run_bass_kernel_spmd(nc, in_maps, core_ids=list(range(8)), trace=trace, **kw)
    ys = [np.asarray(res.results[i]["out"], dtype=np.float32) for i in range(8)]
    y = np.stack([ys[2 * n] + ys[2 * n + 1] for n in range(N)])
    return y.reshape(N, C, 64, 64), res


def kernel(**inputs):
    y, _ = run(inputs, trace=False)
    return y
